# revision 59
# baseline (speedup 1.0000x reference)
"""Multi-head attention (B=2, S=2048, D=2048, H=16) on 8 Trainium2 cores.

Sharding: core = batch (2) x head-group (4 heads each). Tensor-parallel on
wq/wk/wv rows + wo columns; per-core partial outputs summed on host.

Device kernel (per core):
  phase 1 (bf16 inputs, single pass over the 16 contraction chunks):
    qT/kT (head_dim, seq) + v (seq, head_dim) projections, fused RoPE.
    Head 0 streams contraction-outer so PE paces with the x DMA arrivals.
  phase 2+3 (fused): scores^T -> exp(bf16) -> attn@V; softmax denominator
    via DVE running-sum + one ones-matmul per (q-block, head); output
    projection groups interleaved one per pipeline step so PE stays fed
    while ACT runs exp.
"""

import sys

for _p in ("/opt/trn_rl_repo",):
    if _p not in sys.path:
        sys.path.insert(0, _p)

import ml_dtypes
import numpy as np

import concourse.bass as bass
import concourse.bass_isa as bass_isa
import concourse.tile as tile
from concourse import bacc, mybir
from concourse.bass_utils import run_bass_kernel_spmd

F32 = mybir.dt.float32
F32R = mybir.dt.float32r
BF16 = mybir.dt.bfloat16
NP_BF16 = ml_dtypes.bfloat16

DIM = 2048
N_HEADS = 16
HEAD_DIM = 128
BATCH = 2
SEQ = 2048
G_HEADS = 4          # heads per core
GM = G_HEADS * HEAD_DIM  # 512 output cols per core
DC = DIM // 128      # 16 contraction chunks
SC512 = SEQ // 512   # 4
SC128 = SEQ // 128   # 16
INV_SQRT_HD = float(1.0 / np.sqrt(HEAD_DIM))

# even<->odd partition swap within each 32-partition group
_SWAP_MASK = [i ^ 1 for i in range(32)]

Exp = mybir.ActivationFunctionType.Exp
Copy = mybir.ActivationFunctionType.Copy


def build(with_mask: bool):
    nc = bacc.Bacc("TRN2", target_bir_lowering=False, debug=False)

    xt_d = nc.dram_tensor("xt", [DC, 128, SEQ], BF16, kind="ExternalInput").ap()
    wq_d = nc.dram_tensor("wq", [G_HEADS, 128, DC, 128], BF16, kind="ExternalInput").ap()
    wk_d = nc.dram_tensor("wk", [G_HEADS, 128, DC, 128], BF16, kind="ExternalInput").ap()
    wv_d = nc.dram_tensor("wv", [128, DC, GM], BF16, kind="ExternalInput").ap()
    wo_d = nc.dram_tensor("wo", [G_HEADS, 128, SEQ], F32R, kind="ExternalInput").ap()
    ce_d = nc.dram_tensor("ce", [128, SEQ], BF16, kind="ExternalInput").ap()
    s2_d = nc.dram_tensor("s2", [128, SEQ], BF16, kind="ExternalInput").ap()
    ones_d = nc.dram_tensor("ones", [128, 1], BF16, kind="ExternalInput").ap()
    mt_d = None
    if with_mask:
        mt_d = nc.dram_tensor("mt", [SC128, 128, SEQ], F32, kind="ExternalInput").ap()
    out_d = nc.dram_tensor("out", [SC128, 128, SEQ], BF16, kind="ExternalOutput").ap()

    with tile.TileContext(nc) as tc:
        with (
            tc.tile_pool(name="persist", bufs=1) as persist,
            tc.tile_pool(name="consts", bufs=1) as consts,
        ):
            ones_t = consts.tile([128, 1], BF16, tag="ones")
            with tc.tile_wait_until(0.012):
                nc.gpsimd.dma_start(ones_t[:], ones_d)
            # warm the ACT exp LUT early so phase 2 doesn't stall on it
            warm_t = consts.tile([128, 1], BF16, tag="warm")
            nc.scalar.activation(
                out=warm_t[:], in_=ones_t[:], func=Exp,
            )

            q_t = [persist.tile([128, SEQ], BF16, tag=f"q{h}", name=f"q{h}") for h in range(G_HEADS)]
            k_t = [persist.tile([128, SEQ], BF16, tag=f"k{h}", name=f"k{h}") for h in range(G_HEADS)]
            v_t = [persist.tile([128, GM], BF16, tag=f"v{s}", name=f"v{s}") for s in range(SC128)]
            wo_t = [persist.tile([128, SEQ], F32R, tag=f"wo{m}", name=f"wo{m}") for m in range(G_HEADS)]
            ce_t = consts.tile([128, SEQ], BF16, tag="ce")
            s2_t = consts.tile([128, SEQ], BF16, tag="s2")

            # ---------------- phase 1: projections + fused rope ----------------
            with (
                tc.tile_pool(name="xt", bufs=DC) as xt_pool,
                tc.tile_pool(name="wqk", bufs=4) as wqk_pool,
                tc.tile_pool(name="wv", bufs=1) as wv_pool,
                tc.tile_pool(name="ps1", bufs=8, space="PSUM") as ps1,
                tc.tile_pool(name="rope_t", bufs=2) as rope_t,
            ):
                # startup: first head's weights + first x tiles get scheduler
                # priority, and are split into small transfers so the first
                # matmuls start ~2.5us in instead of waiting for full tiles
                # (the DMA transfer resource is serial)
                xts = []
                with tc.high_priority():
                    # the HWDGE generator is a single serial resource shared
                    # by the SP/ACT queues (~630ns per DMA); the first weight
                    # quarters take its first two slots while x0/x1 ride the
                    # SWDGE (gpsimd) path that bypasses it
                    wtq0 = wqk_pool.tile([128, DC, 128], BF16, tag="w", name="wtq0")
                    nc.sync.dma_start(wtq0[:, 0:4, :], wq_d[0][:, 0:4, :])
                    wtk0 = wqk_pool.tile([128, DC, 128], BF16, tag="w", name="wtk0")
                    nc.scalar.dma_start(wtk0[:, 0:4, :], wk_d[0][:, 0:4, :])
                    for i in range(2):
                        x = xt_pool.tile([128, SEQ], BF16, tag="x", name=f"x{i}")
                        nc.gpsimd.dma_start(x[:, 0:1024], xt_d[i][:, 0:1024])
                        nc.gpsimd.dma_start(x[:, 1024:2048], xt_d[i][:, 1024:2048])
                        xts.append(x)
                for i in range(2, DC):
                    x = xt_pool.tile([128, SEQ], BF16, tag="x", name=f"x{i}")
                    eng = (nc.sync, nc.scalar)[i % 2]
                    if i < 6:
                        # halves let head 0's paired sc0/sc1 matmuls start as
                        # soon as the first 2KB/partition lands
                        eng.dma_start(x[:, 0:1024], xt_d[i][:, 0:1024])
                        eng.dma_start(x[:, 1024:2048], xt_d[i][:, 1024:2048])
                    else:
                        eng.dma_start(x[:], xt_d[i])
                    xts.append(x)
                # later weight quarters ride behind the x stream (chunk i of
                # head 0 needs quarter i//4 only ~i*1.7us in)
                for qtr in range(1, 4):
                    cs = slice(4 * qtr, 4 * qtr + 4)
                    with tc.tile_wait_until(0.002 * qtr):
                        nc.gpsimd.dma_start(wtq0[:, cs, :], wq_d[0][:, cs, :])
                        nc.gpsimd.dma_start(wtk0[:, cs, :], wk_d[0][:, cs, :])
                # rope tables: needed from the first head's copies (~28us);
                # wait_until keeps them from jumping ahead of the x stream
                with tc.tile_wait_until(0.022):
                    nc.gpsimd.dma_start(ce_t[:], ce_d)
                    nc.gpsimd.dma_start(s2_t[:], s2_d)
                wvt = wv_pool.tile([128, DC, GM], BF16, tag="wv", name="wvt")
                with tc.tile_wait_until(0.028):
                    nc.scalar.dma_start(wvt[:], wv_d)
                with tc.tile_wait_until(0.055):
                    for m in range(G_HEADS):
                        nc.gpsimd.dma_start(wo_t[m][:], wo_d[m])

                def rope(t, sl):
                    # sin-product on the otherwise-idle GpSimd engine
                    t1 = rope_t.tile([128, 512], BF16, tag="t1", name="t1")
                    nc.gpsimd.tensor_mul(out=t1[:], in0=t[:, sl], in1=s2_t[:, sl])
                    t2 = rope_t.tile([128, 512], BF16, tag="t2", name="t2")
                    nc.vector.stream_shuffle(t2[:], t1[:], _SWAP_MASK)
                    t3 = rope_t.tile([128, 512], BF16, tag="t3", name="t3")
                    nc.vector.tensor_mul(out=t3[:], in0=t[:, sl], in1=ce_t[:, sl])
                    nc.vector.tensor_add(out=t[:, sl], in0=t3[:], in1=t2[:])

                # head 0: contraction-outer so the 8 open PSUM groups pace
                # with x-tile DMA arrivals instead of waiting for all 16;
                # sc-inner q/k pairing matches the half-tile DMA splits
                psq = [ps1.tile([128, 512], F32, tag="ps", name=f"psq{sc}") for sc in range(SC512)]
                psk = [ps1.tile([128, 512], F32, tag="ps", name=f"psk{sc}") for sc in range(SC512)]
                for i in range(DC):
                    for sc in range(SC512):
                        nc.tensor.matmul(
                            psq[sc][:], wtq0[:, i, :], xts[i][:, bass.ts(sc, 512)],
                            start=(i == 0), stop=(i == DC - 1),
                        )
                        nc.tensor.matmul(
                            psk[sc][:], wtk0[:, i, :], xts[i][:, bass.ts(sc, 512)],
                            start=(i == 0), stop=(i == DC - 1),
                        )
                for dst, pss in ((q_t[0], psq), (k_t[0], psk)):
                    for sc in range(SC512):
                        sl = bass.ts(sc, 512)
                        nc.scalar.activation(out=dst[:, sl], in_=pss[sc][:], func=Copy)
                        rope(dst, sl)

                # v projection groups, interleaved through heads 1..3
                v_next = 0

                def emit_v(n, on_dve=False):
                    nonlocal v_next
                    for s in range(v_next, v_next + n):
                        ps = ps1.tile([128, GM], F32, tag="ps", name="psv")
                        for i in range(DC):
                            nc.tensor.matmul(
                                ps[:], xts[i][:, bass.ts(s, 128)], wvt[:, i, :],
                                start=(i == 0), stop=(i == DC - 1),
                            )
                        if on_dve:
                            nc.vector.tensor_copy(out=v_t[s][:], in_=ps[:])
                        else:
                            nc.scalar.activation(out=v_t[s][:], in_=ps[:], func=Copy)
                    v_next += n

                V_SCHED = {(1, 0): 2, (1, 1): 3, (2, 0): 3, (2, 1): 3, (3, 0): 3, (3, 1): 2}
                # weight loads staggered to land a few us before each block
                W_WAIT = {(1, 0): 0.024, (1, 1): 0.040, (2, 0): 0.058,
                          (2, 1): 0.075, (3, 0): 0.095, (3, 1): 0.112}
                for h in range(1, G_HEADS):
                    for wi, (wd, dst) in enumerate(((wq_d, q_t[h]), (wk_d, k_t[h]))):
                        wt = wqk_pool.tile([128, DC, 128], BF16, tag="w", name="wt")
                        with tc.tile_wait_until(W_WAIT[(h, wi)]):
                            nc.sync.dma_start(wt[:], wd[h])
                        for sc in range(SC512):
                            ps = ps1.tile([128, 512], F32, tag="ps", name="ps")
                            for i in range(DC):
                                nc.tensor.matmul(
                                    ps[:], wt[:, i, :], xts[i][:, bass.ts(sc, 512)],
                                    start=(i == 0), stop=(i == DC - 1),
                                )
                            sl = bass.ts(sc, 512)
                            if h == 3:
                                # last block's copies on DVE: keeps the ACT
                                # queue clear so ic0's first exps start at the
                                # transition instead of behind these
                                nc.vector.tensor_copy(out=dst[:, sl], in_=ps[:])
                            else:
                                nc.scalar.activation(out=dst[:, sl], in_=ps[:], func=Copy)
                            rope(dst, sl)
                        emit_v(V_SCHED[(h, wi)], on_dve=(h == 3))

            # ---------------- phase 2+3 (fused) ----------------
            o_pool = tc.alloc_tile_pool(name="oT", bufs=1)
            o_t = [o_pool.tile([128, SEQ], F32R, tag=f"o{h}", name=f"o{h}") for h in range(G_HEADS)]
            with (
                tc.tile_pool(name="est", bufs=5) as est_pool,
                tc.tile_pool(name="ehp", bufs=6) as eh_pool,
                tc.tile_pool(name="nrm", bufs=4) as nrm_pool,
                tc.tile_pool(name="fin", bufs=6) as fin_pool,
                tc.tile_pool(name="ps_st", bufs=2, space="PSUM") as ps_st,
                tc.tile_pool(name="ps_av", bufs=3, space="PSUM") as ps_av,
                tc.tile_pool(name="ps3", bufs=1, space="PSUM") as ps3,
            ):
                if with_mask:
                    mask_pool = tc.alloc_tile_pool(name="mask", bufs=1)

                p3q = []

                p3_ctr = [0]

                def emit_p3(pool, n=1):
                    for _ in range(n):
                        if not p3q:
                            return
                        s, nck = p3q.pop(0)
                        ps = pool.tile([128, 512], F32, tag="p3", name="p3")
                        ssl = bass.ts(s, 128)
                        nsl = bass.ts(nck, 512)
                        for mc in range(G_HEADS):
                            nc.tensor.matmul(
                                ps[:], o_t[mc][:, ssl], wo_t[mc][:, nsl],
                                start=(mc == 0), stop=(mc == G_HEADS - 1),
                            )
                        f = fin_pool.tile([128, 512], BF16, tag="f", name="f")
                        nc.vector.tensor_copy(out=f[:], in_=ps[:])
                        # in-pipeline groups issue on SP only: a DMA on the
                        # ACT sequencer costs 667ns of exp-dispatch time and
                        # the pipeline's ACT margin is thinner than that
                        if pool is ps3:
                            eng = nc.sync
                        else:
                            eng = (nc.sync, nc.scalar)[p3_ctr[0] % 2]
                        p3_ctr[0] += 1
                        eng.dma_start(out_d[s, :, nsl], f[:])

                for ic in range(SC512):
                    isl = bass.ts(ic, 512)
                    for hp in range(G_HEADS // 2):
                        heads = (2 * hp, 2 * hp + 1)
                        acc = {}
                        ehacc = {}
                        e_of = {}
                        m_of = {}
                        for h in heads:
                            acc[h] = ps_av.tile([128, 512], F32, tag="acc", name="acc")

                        def emit_st(h, jc2):
                            ja, jb = 2 * jc2, 2 * jc2 + 1
                            st = ps_st.tile([128, 1024], F32, tag="st", name="st")
                            nc.tensor.matmul(
                                st[:, 0:512],
                                k_t[h][:, bass.ts(ja, 128)], q_t[h][:, isl],
                                start=True, stop=True,
                            )
                            nc.tensor.matmul(
                                st[:, 512:1024],
                                k_t[h][:, bass.ts(jb, 128)], q_t[h][:, isl],
                                start=True, stop=True,
                            )
                            e = est_pool.tile([128, 1024], BF16, tag="e", name="e")
                            if with_mask:
                                if jc2 not in m_of:
                                    mtl = mask_pool.tile(
                                        [128, 1024], F32, tag="m", name="mtl"
                                    )
                                    nc.sync.dma_start(mtl[:, 0:512], mt_d[ja, :, isl])
                                    nc.sync.dma_start(mtl[:, 512:1024], mt_d[jb, :, isl])
                                    m_of[jc2] = mtl
                                es = est_pool.tile([128, 1024], BF16, tag="es", name="es", bufs=2)
                                nc.vector.tensor_add(
                                    out=es[:], in0=st[:], in1=m_of[jc2][:]
                                )
                                nc.scalar.activation(out=e[:], in_=es[:], func=Exp)
                            else:
                                nc.scalar.activation(out=e[:], in_=st[:], func=Exp)
                            # fold the two j-chunk halves + running denominator
                            # sum on DVE (bf16, 2x mode) - frees PE from the
                            # per-chunk ones-matmuls
                            if jc2 == 0:
                                ea = eh_pool.tile([128, 512], BF16, tag="ea", name="ea")
                                nc.vector.tensor_add(
                                    out=ea[:], in0=e[:, 0:512], in1=e[:, 512:1024]
                                )
                                ehacc[h] = ea
                            else:
                                eh = eh_pool.tile([128, 512], BF16, tag="eh", name="eh")
                                nc.vector.tensor_add(
                                    out=eh[:], in0=e[:, 0:512], in1=e[:, 512:1024]
                                )
                                nc.vector.tensor_add(
                                    out=ehacc[h][:], in0=ehacc[h][:], in1=eh[:]
                                )
                            e_of[(h, jc2)] = e

                        def emit_av(h, jc2):
                            ja, jb = 2 * jc2, 2 * jc2 + 1
                            e = e_of.pop((h, jc2))
                            last = jc2 == SC128 // 2 - 1
                            nc.tensor.matmul(
                                acc[h][:], v_t[ja][:, bass.ts(h, 128)], e[:, 0:512],
                                start=(jc2 == 0), stop=False,
                            )
                            nc.tensor.matmul(
                                acc[h][:], v_t[jb][:, bass.ts(h, 128)], e[:, 512:1024],
                                start=False, stop=last,
                            )

                        def finish(h):
                            # denominator: GpSimd all-reduce over partitions
                            # (f32 accumulate) - no PE ones-matmul, and every
                            # partition gets the sum so no broadcast hop
                            ar = nrm_pool.tile([128, 512], F32, tag="ar", name="ar")
                            nc.gpsimd.partition_all_reduce(
                                ar[:], ehacc[h][:], 128, bass_isa.ReduceOp.add
                            )
                            rec = nrm_pool.tile([128, 512], F32, tag="rec", name="rec")
                            nc.vector.reciprocal_approx_fast(out=rec[:], in_=ar[:])
                            nc.vector.tensor_mul(
                                out=o_t[h][:, isl], in0=acc[h][:], in1=rec[:]
                            )

                        # software pipeline: the AV stream runs one full step
                        # behind the ST stream so every av consumes an exp
                        # that finished a step ago; output-projection groups
                        # fill the remaining PE slack.  In hp0 the first p3
                        # steps are skipped (o_t of the previous q-block is
                        # still being written by DVE); hp1 injects from the
                        # prologue on.
                        h0, h1 = heads
                        NJ2 = SC128 // 2
                        if hp == 0:
                            P3N = [0, 0, 1, 1, 1, 1, 2, 2]
                        else:
                            P3N = [1, 1, 1, 1, 1, 1, 1, 0]
                        emit_st(h0, 0)
                        emit_st(h1, 0)
                        if hp == 1:
                            emit_p3(ps3)
                        for jc2 in range(NJ2):
                            if jc2 + 1 < NJ2:
                                emit_st(h0, jc2 + 1)
                            emit_av(h0, jc2)
                            emit_p3(ps3, P3N[jc2])
                            if jc2 + 1 < NJ2:
                                emit_st(h1, jc2 + 1)
                            emit_av(h1, jc2)
                        finish(h0)
                        finish(h1)

                    for s in range(ic * SC512, (ic + 1) * SC512):
                        for nck in range(SC512):
                            p3q.append((s, nck))
                if with_mask:
                    mask_pool.release()
            # drain the last q-block's projection groups through a wider
            # PSUM pool (the attention pools above are closed by now)
            with (
                tc.tile_pool(name="fin2", bufs=8) as fin_pool,
                tc.tile_pool(name="ps3b", bufs=4, space="PSUM") as ps3b,
            ):
                # the last q-block's o2/o3 normalize trails its o0/o1 by a
                # few us: open the first groups on the early-ready heads and
                # close them once the late heads land
                opened = []
                for _ in range(min(4, len(p3q))):
                    s, nck = p3q.pop(0)
                    ps = ps3b.tile([128, 512], F32, tag="p3", name="p3")
                    for mc in (0, 1):
                        nc.tensor.matmul(
                            ps[:], o_t[mc][:, bass.ts(s, 128)],
                            wo_t[mc][:, bass.ts(nck, 512)],
                            start=(mc == 0), stop=False,
                        )
                    opened.append((ps, s, nck))
                for ps, s, nck in opened:
                    for mc in (2, 3):
                        nc.tensor.matmul(
                            ps[:], o_t[mc][:, bass.ts(s, 128)],
                            wo_t[mc][:, bass.ts(nck, 512)],
                            start=False, stop=(mc == 3),
                        )
                    f = fin_pool.tile([128, 512], BF16, tag="f", name="f")
                    nc.vector.tensor_copy(out=f[:], in_=ps[:])
                    eng = (nc.sync, nc.scalar)[p3_ctr[0] % 2]
                    p3_ctr[0] += 1
                    eng.dma_start(out_d[s, :, bass.ts(nck, 512)], f[:])
                while p3q:
                    emit_p3(ps3b)
            o_pool.release()

    nc.compile()
    return nc


_CACHE = {}


def _get_nc(with_mask: bool):
    if with_mask not in _CACHE:
        _CACHE[with_mask] = build(with_mask)
    return _CACHE[with_mask]


def kernel(in_token, freqs_cos, freqs_sin, mask, wq, wk, wv, wo):
    return _run(in_token, freqs_cos, freqs_sin, mask, wq, wk, wv, wo)


def run_traced(in_token, freqs_cos, freqs_sin, mask, wq, wk, wv, wo):
    """Test-only: run with NTFF tracing, return (output, BassKernelResults)."""
    return _run(in_token, freqs_cos, freqs_sin, mask, wq, wk, wv, wo, trace=True)


def _run(in_token, freqs_cos, freqs_sin, mask, wq, wk, wv, wo, trace=False):
    in_token = np.asarray(in_token, dtype=np.float32)
    freqs_cos = np.asarray(freqs_cos, dtype=np.float32)
    freqs_sin = np.asarray(freqs_sin, dtype=np.float32)
    mask = np.asarray(mask, dtype=np.float32)
    wq = np.asarray(wq, dtype=np.float32)
    wk = np.asarray(wk, dtype=np.float32)
    wv = np.asarray(wv, dtype=np.float32)
    wo = np.asarray(wo, dtype=np.float32)

    with_mask = bool(np.any(mask))
    nc = _get_nc(with_mask)

    # rope tables in (head_dim, seq) pair-expanded layout, signs/swap baked in
    ce = np.repeat(freqs_cos.T, 2, axis=0).astype(NP_BF16)  # (128, S)
    s2 = np.empty((HEAD_DIM, SEQ), np.float32)
    s2[0::2] = freqs_sin.T   # even rows: +sin (lands on odd out after swap)
    s2[1::2] = -freqs_sin.T  # odd rows: -sin (lands on even out after swap)
    s2 = s2.astype(NP_BF16)
    ones = np.ones((128, 1), NP_BF16)
    if with_mask:
        mt = np.ascontiguousarray(mask.T).reshape(SC128, 128, SEQ)

    in_maps = []
    xts = [
        np.ascontiguousarray(in_token[b].T.astype(NP_BF16)).reshape(DC, 128, SEQ)
        for b in range(BATCH)
    ]
    for b in range(BATCH):
        for g in range(G_HEADS):
            rows = slice(g * GM, (g + 1) * GM)
            wqt = np.ascontiguousarray(
                (wq[rows] * INV_SQRT_HD).T.reshape(
                    DC, 128, G_HEADS, 128
                ).transpose(2, 1, 0, 3).astype(NP_BF16)
            )
            wkt = np.ascontiguousarray(
                wk[rows].T.reshape(DC, 128, G_HEADS, 128)
                .transpose(2, 1, 0, 3).astype(NP_BF16)
            )
            wvt = np.ascontiguousarray(
                wv[rows].T.reshape(DC, 128, GM).transpose(1, 0, 2).astype(NP_BF16)
            )
            wot = np.ascontiguousarray(wo[:, rows].T).reshape(G_HEADS, 128, SEQ)
            m = {
                "xt": xts[b], "wq": wqt, "wk": wkt, "wv": wvt, "wo": wot,
                "ce": ce, "s2": s2, "ones": ones,
            }
            if with_mask:
                m["mt"] = mt
            in_maps.append(m)

    res = run_bass_kernel_spmd(nc, in_maps, core_ids=list(range(8)), trace=trace)

    out = np.zeros((BATCH, SEQ, DIM), np.float32)
    for b in range(BATCH):
        acc = None
        for g in range(G_HEADS):
            p = res.results[b * G_HEADS + g]["out"].astype(np.float32).reshape(SEQ, DIM)
            acc = p if acc is None else acc + p
        out[b] = acc
    if trace:
        return out, res
    return out
